# revision 39
# baseline (speedup 1.0000x reference)
"""Trainium2 Bass kernel for nn_DTCSTGCN (dynamic spatio-temporal GCN).

Sharding: pure data-parallel over batch. B=8 == 8 NeuronCores, one batch
element per core, weights replicated, no collectives.

Algebraic structure exploited (exact reassociations of the reference):
  * term2 @ term3 = (f_prev ⊗ (Ws@Wp)) @ (Wa@Ap) collapses to
    f_prev ⊗ (v@Ap) with v = Ws@Wp@Wa  -> kills 3 of 4 NxN matmuls.
  * sigmoid(pre) is built directly in TRANSPOSED layout (it is a rank-1
    argument), so the one remaining big matmul sig @ Vs needs no
    transposes.
  * At = D S D (S = exp(-d2) * mask symmetric, D = diag(deg^-1/2)) is
    never materialized: the D factors fold into cheap per-row/column
    scalings around the S matmuls (lrelu commutes with positive scaling).
  * GCN layer 1: A_att @ (x W1 + b1) is rank-2 -> two row-reductions.
All heavy matmuls run in float32r (full PE rate, ~1e-4 rounding).
"""
import numpy as np

import concourse.bass as bass
import concourse.mybir as mybir
import concourse.tile as tile
from concourse.bass_utils import run_bass_kernel_spmd
from concourse.masks import make_identity

P = 128
N = 512
NC = N // P            # 4 partition chunks
T = 24
GH, GO, LH = 32, 64, 64
COMB = GO + LH         # 128
HYB1, HYB2, PRED = 32, 16, 3
F32 = mybir.dt.float32
F32R = mybir.dt.float32r
AF = mybir.ActivationFunctionType
ALU = mybir.AluOpType
AX = mybir.AxisListType

_wsplit_counter = [0]


def _split_excess_waits(nc):
    """This walrus build accepts at most ONE sync wait per instruction
    (two for EventSemaphore); Tile emits more. Spill extras onto NoOps
    inserted before the instruction on the same engine (queues execute
    in order, so semantics are unchanged)."""
    for func in nc.m.functions:
        for bb in func.blocks:
            out = []
            changed = False
            for ins in bb.instructions:
                si = ins.sync_info
                cap = 2 if isinstance(ins, mybir.InstEventSemaphore) else 1
                if si is not None and si.on_wait and len(si.on_wait) > cap:
                    waits = list(si.on_wait)
                    extras, keep = waits[:-cap], waits[-cap:]
                    for w in extras:
                        _wsplit_counter[0] += 1
                        nop = mybir.InstNoOp(
                            name=f"I-wsplit-{_wsplit_counter[0]}",
                            ins=[], outs=[])
                        nop.engine = ins.engine
                        nop.sync_info = mybir.SyncInfo(
                            on_wait=[w], on_update=[])
                        out.append(nop)
                    ins.sync_info = mybir.SyncInfo(
                        on_wait=keep, on_update=list(si.on_update))
                    changed = True
                out.append(ins)
            if changed:
                bb.instructions = out


def _emit(nc, flags):
    """Emit the full per-core graph. flags: which bias inputs are nonzero."""
    dt_in = {}

    def inp(name, shape):
        dt_in[name] = nc.dram_tensor(name, shape, F32, kind="ExternalInput")
        return dt_in[name]

    xb = inp("xb", [T, N])
    adjm = inp("adjm", [N, N])
    adjn = inp("adjn", [N, N])
    Vs = inp("Vs", [N, N])
    Ws = inp("Ws", [1, N])
    Wp = inp("Wp", [N, N])
    Wa = inp("Wa", [N, N])
    W1 = inp("W1", [1, GH])
    W2 = inp("W2", [GH, GH])
    W3 = inp("W3", [GH, GO])
    W_ih = inp("W_ih", [4 * LH, 1])
    W_hh = inp("W_hh", [4 * LH, LH])
    b_ih = inp("b_ih", [4 * LH])
    b_hh = inp("b_hh", [4 * LH])
    W_attn = inp("W_attn", [COMB, COMB])
    Wh1 = inp("Wh1", [COMB, HYB1])
    Wh2 = inp("Wh2", [HYB1, HYB2])
    W_fc = inp("W_fc", [HYB2, PRED])
    if flags["bs"]:
        bs = inp("bs", [N, N])
    if flags["b1"]:
        b1 = inp("b1", [GH])
    if flags["b2"]:
        b2 = inp("b2", [GH])
    if flags["b3"]:
        b3 = inp("b3", [GO])
    if flags["b_attn"]:
        b_attn = inp("b_attn", [COMB])
    if flags["bh1"]:
        bh1 = inp("bh1", [HYB1])
    if flags["bh2"]:
        bh2 = inp("bh2", [HYB2])
    if flags["b_fc"]:
        b_fc = inp("b_fc", [PRED])
    out_d = nc.dram_tensor("out", [PRED, N], F32, kind="ExternalOutput")

    tc = tile.TileContext(nc)
    tc.__enter__()
    import contextlib
    ctx = contextlib.ExitStack()
    # ---------------- pools ----------------
    cst = ctx.enter_context(tc.tile_pool(name="cst", bufs=1))
    big = ctx.enter_context(tc.tile_pool(name="big", bufs=2))    # [128,4,512] per-t
    big1 = ctx.enter_context(tc.tile_pool(name="big1", bufs=1))
    row = ctx.enter_context(tc.tile_pool(name="row", bufs=1))    # [1,512] rows
    sm = ctx.enter_context(tc.tile_pool(name="sm", bufs=2))      # small per-t
    st = ctx.enter_context(tc.tile_pool(name="st", bufs=1))      # persistent state
    # PSUM budget: 8 banks total. bufs applies PER TAG:
    # spre x2 + pbc x2 + (pz, pga, pgb) x1 + pmisc x1 = 8 banks.
    pp_spre = ctx.enter_context(tc.tile_pool(name="pp_spre", bufs=2, space="PSUM"))
    pp_bc = ctx.enter_context(tc.tile_pool(name="pp_bc", bufs=2, space="PSUM"))
    pp_g = ctx.enter_context(tc.tile_pool(name="pp_g", bufs=1, space="PSUM"))
    pp_m = ctx.enter_context(tc.tile_pool(name="pp_m", bufs=1, space="PSUM"))

    def rr(h):  # [N, M] dram -> [P, NC, M] chunked over rows
        return h.rearrange("(c p) m -> p c m", p=P)

    # ---------------- constants / setup ----------------
    ident = cst.tile([P, P], F32)
    make_identity(nc, ident)
    ones1r = cst.tile([1, P], F32R)
    nc.vector.memset(ones1r.bitcast(F32), 1.0)

    # bigmask = 1e4 * (1 - mask): added to d2 so exp(-d2') underflows to
    # exactly 0 where mask==0 (mask entries are 0/1) -> S = exp(-d2')
    # directly, with deg as the exp's accum_out.
    mask_stage = cst.tile([P, NC, N], F32, tag="stage")
    nc.sync.dma_start(out=mask_stage, in_=rr(adjm))
    bigmask = cst.tile([P, NC, N], F32)
    nc.vector.tensor_scalar(bigmask, mask_stage, -1.0, -1e4,
                            ALU.add, ALU.mult)
    stage = cst.tile([P, NC, N], F32, tag="stage")
    Vs_r = cst.tile([P, NC, N], F32R)
    nc.sync.dma_start(out=stage, in_=rr(Vs))
    nc.vector.tensor_copy(Vs_r, stage)
    stage2 = cst.tile([P, NC, N], F32, tag="stage")
    adjn_r = cst.tile([P, NC, N], F32R)
    nc.sync.dma_start(out=stage2, in_=rr(adjn))
    nc.vector.tensor_copy(adjn_r, stage2)

    # x transposed (columns); rows are DMA'd per-timestep (partition-0
    # slices -- engines cannot address partition offsets not mult. of 32)
    xT = cst.tile([P, NC, T], F32)
    for c in range(NC):
        nc.sync.dma_start(out=xT[:, c, :],
                          in_=xb[:, c * P:(c + 1) * P].rearrange("t p -> p t"))
    xTn = cst.tile([P, NC, T], F32)
    nc.vector.tensor_scalar_mul(xTn, xT, -1.0)

    # small weights
    W2s = cst.tile([GH, GH], F32R)
    W3s = cst.tile([GH, GO], F32R)
    W1row = cst.tile([1, GH], F32R)
    tmpw2 = cst.tile([GH, GH], F32, tag="wstage")
    nc.sync.dma_start(out=tmpw2, in_=W2[:, :])
    nc.vector.tensor_copy(W2s, tmpw2)
    tmpw3 = cst.tile([GH, GO], F32, tag="wstage")
    nc.sync.dma_start(out=tmpw3, in_=W3[:, :])
    nc.vector.tensor_copy(W3s, tmpw3)
    tmpw1 = cst.tile([1, GH], F32, tag="wstage")
    nc.sync.dma_start(out=tmpw1, in_=W1[:, :])
    nc.vector.tensor_copy(W1row, tmpw1)

    Wattn_r = cst.tile([COMB, COMB], F32R)
    tmpwa = cst.tile([COMB, COMB], F32, tag="wstage")
    nc.sync.dma_start(out=tmpwa, in_=W_attn[:, :])
    nc.vector.tensor_copy(Wattn_r, tmpwa)
    Wh1_r = cst.tile([COMB, HYB1], F32R)
    tmpwh1 = cst.tile([COMB, HYB1], F32, tag="wstage")
    nc.sync.dma_start(out=tmpwh1, in_=Wh1[:, :])
    nc.vector.tensor_copy(Wh1_r, tmpwh1)
    Wh2_r = cst.tile([HYB1, HYB2], F32R)
    tmpwh2 = cst.tile([HYB1, HYB2], F32, tag="wstage")
    nc.sync.dma_start(out=tmpwh2, in_=Wh2[:, :])
    nc.vector.tensor_copy(Wh2_r, tmpwh2)
    # pad PRED 3 -> 4 (f32r matmul dst pattern requires even/aligned free)
    PREDP = 4
    Wfc_r = cst.tile([HYB2, PREDP], F32R)
    tmpwfc = cst.tile([HYB2, PREDP], F32, tag="wstage")
    nc.vector.memset(tmpwfc, 0.0)
    nc.sync.dma_start(out=tmpwfc[:, 0:PRED], in_=W_fc[:, :])
    nc.vector.tensor_copy(Wfc_r, tmpwfc)

    # optional bias rows
    def bias_row(h, n, nm):
        t0 = cst.tile([1, n], F32, tag="wstage")
        nc.sync.dma_start(out=t0, in_=h.unsqueeze(0))
        tr_ = cst.tile([1, n], F32R, tag=nm)
        nc.vector.tensor_copy(tr_, t0)
        return tr_

    b2row = bias_row(b2, GH, "b2r") if flags["b2"] else None
    b3row = bias_row(b3, GO, "b3r") if flags["b3"] else None
    battnrow = bias_row(b_attn, COMB, "bar") if flags["b_attn"] else None
    bh1row = bias_row(bh1, HYB1, "bh1r") if flags["bh1"] else None
    bh2row = bias_row(bh2, HYB2, "bh2r") if flags["bh2"] else None
    bfcrow4 = None
    if flags["b_fc"]:
        t0 = cst.tile([1, 4], F32, tag="wstage")
        nc.vector.memset(t0, 0.0)
        nc.sync.dma_start(out=t0[:, 0:PRED], in_=b_fc.unsqueeze(0))
        bfcrow4 = cst.tile([1, 4], F32R, tag="bfcr")
        nc.vector.tensor_copy(bfcrow4, t0)
    b1row = bias_row(b1, GH, "b1r") if flags["b1"] else None

    # bs transposed (only if nonzero)
    bsT = None
    if flags["bs"]:
        bs_stage = cst.tile([P, NC, N], F32, tag="stage")
        nc.sync.dma_start(out=bs_stage, in_=rr(bs))
        bsT = cst.tile([P, NC, N], F32)
        for i in range(NC):
            for j in range(NC):
                pt = pp_m.tile([P, P], F32, tag="pmisc")
                nc.tensor.transpose(pt, bs_stage[:, i, j * P:(j + 1) * P], ident)
                nc.vector.tensor_copy(bsT[:, j, i * P:(i + 1) * P], pt)

    # LSTM fused weights Wcat [128, 2, 128]:
    # row 0 = W_ih, row 32 = bsum, rows 64..127 = W_hh^T, rest zero pad.
    # (layout keeps every engine access at a legal partition base/span)
    KL = 128
    wcat_f = cst.tile([KL, 2, P], F32)
    nc.vector.memset(wcat_f[0:KL], 0.0)
    nc.sync.dma_start(out=wcat_f[0:1], in_=W_ih.rearrange("(c g) o -> o c g", c=2))
    nc.sync.dma_start(out=wcat_f[64:64 + LH],
                      in_=W_hh.rearrange("(c g) h -> h c g", c=2))
    bstg = cst.tile([1, 2, P], F32, tag="wstage")
    bstg2 = cst.tile([1, 2, P], F32, tag="wstage2")
    nc.sync.dma_start(out=bstg, in_=b_ih.rearrange("(c g) -> c g", c=2).unsqueeze(0))
    nc.sync.dma_start(out=bstg2, in_=b_hh.rearrange("(c g) -> c g", c=2).unsqueeze(0))
    nc.vector.tensor_add(wcat_f[32:33], bstg, bstg2)
    wcat_r = cst.tile([KL, 2, P], F32R)
    nc.vector.tensor_copy(wcat_r, wcat_f)

    # v = Ws @ Wp @ Wa  (on device)
    wp_sb = cst.tile([P, NC, N], F32, tag="stage")
    nc.sync.dma_start(out=wp_sb, in_=rr(Wp))
    ws_col = cst.tile([P, NC, 1], F32)
    nc.sync.dma_start(out=ws_col, in_=Ws.rearrange("o (c p) -> p c o", p=P))
    ps_r1 = pp_m.tile([1, N], F32, tag="pmisc")
    for kc in range(NC):
        nc.tensor.matmul(ps_r1, ws_col[:, kc, :], wp_sb[:, kc, :],
                         start=(kc == 0), stop=(kc == NC - 1))
    r1_sb = cst.tile([1, N], F32)
    nc.vector.tensor_copy(r1_sb, ps_r1)
    r1_col = cst.tile([P, NC, 1], F32)
    for c in range(NC):
        pc = pp_m.tile([P, 1], F32, tag="pmisc")
        nc.tensor.transpose(pc, r1_sb[:, c * P:(c + 1) * P], ident[0:1, 0:1])
        nc.vector.tensor_copy(r1_col[:, c, :], pc)
    wa_sb = cst.tile([P, NC, N], F32, tag="stage")
    nc.sync.dma_start(out=wa_sb, in_=rr(Wa))
    ps_v = pp_m.tile([1, N], F32, tag="pmisc")
    for kc in range(NC):
        nc.tensor.matmul(ps_v, r1_col[:, kc, :], wa_sb[:, kc, :],
                         start=(kc == 0), stop=(kc == NC - 1))
    v_row = cst.tile([1, N], F32)
    nc.vector.tensor_copy(v_row, ps_v)

    # LSTM state: RCAT rows: 0=x_t, 32=ones, 64..127=h, rest pad(0)
    RCAT = st.tile([KL, N], F32R)
    nc.vector.memset(RCAT.bitcast(F32)[0:KL], 0.0)
    onesf = cst.tile([1, N], F32, tag="wstage")
    nc.vector.memset(onesf, 1.0)
    nc.vector.tensor_copy(RCAT[32:33, :], onesf)
    c_st = st.tile([LH, N], F32)
    nc.vector.memset(c_st, 0.0)
    HsT = st.tile([GO, N], F32)
    nc.vector.memset(HsT, 0.0)

    # ---------------- per-timestep loop ----------------
    prev = {}  # tiles carried t-1 -> t
    for t in range(T):
        # --- f[t] row (DMA to partition 0) ---
        ftile = sm.tile([1, N], F32, tag="ftile")
        nc.sync.dma_start(out=ftile, in_=xb[t:t + 1, :])
        f_r = sm.tile([1, N], F32R, tag="f_r")
        nc.vector.tensor_copy(f_r, ftile)

        # --- broadcast f[t] ---
        pbc_f = pp_bc.tile([P, N], F32, tag="pbc")
        nc.tensor.matmul(pbc_f, ones1r, f_r, start=True, stop=True)

        # --- d2' = (fj-fi)^2 + bigmask ---
        d2 = big.tile([P, NC, N], F32, tag="d2")
        for c in range(NC):
            nc.scalar.activation(d2[:, c, :], pbc_f, AF.Square,
                                 bias=xTn[:, c, t:t + 1], scale=1.0)
        nc.vector.tensor_add(d2, d2, bigmask)

        # keep f-bcast for next step's sigT
        fbc_sb = sm.tile([P, N], F32, tag="fbc")
        nc.vector.tensor_copy(fbc_sb, pbc_f)

        # --- S = exp(-d2') (f32r) with deg = rowsum(S) fused ---
        S = big.tile([P, NC, N], F32R, tag="S")
        deg = sm.tile([P, NC], F32, tag="deg")
        for c in range(NC):
            nc.scalar.activation(S[:, c, :], d2[:, c, :], AF.Exp,
                                 scale=-1.0, accum_out=deg[:, c:c + 1])
        Sf = S.bitcast(F32)

        # --- dis = deg^-1/2 (diag > 0 always: adj_mask has self-loops) ---
        dsq = sm.tile([P, NC], F32, tag="dsq")
        nc.scalar.activation(dsq, deg, AF.Sqrt)
        dis_c = sm.tile([P, NC], F32, tag="dis_c")
        nc.vector.reciprocal(dis_c, dsq)
        # dis as row [1,512]: per-column PE transposes ([128,1] -> [1,128]
        # at partition 0; offsets 1-3 would be illegal partition accesses)
        dis_row = row.tile([1, NC, P], F32R, tag="dis_row")
        for c in range(NC):
            ptd = pp_m.tile([1, P], F32, tag="pmisc")
            nc.tensor.transpose(ptd, dis_c[:, c:c + 1], ident)
            nc.vector.tensor_copy(dis_row[:, c, :], ptd)
        dis_rowf = dis_row.bitcast(F32)
        dis_row1 = dis_row.rearrange("o c p -> o (c p)")
        # dis broadcast [128, 512]
        pbc_d = pp_bc.tile([P, N], F32, tag="pbc")
        nc.tensor.matmul(pbc_d, ones1r, dis_row1, start=True, stop=True)
        dis_bc = sm.tile([P, N], F32, tag="dis_bc")
        nc.vector.tensor_copy(dis_bc, pbc_d)

        # --- xd = f[t]*dis, vd = v*dis rows; broadcasts ---
        xd_row = row.tile([1, N], F32R, tag="xd_row")
        nc.vector.tensor_mul(xd_row, ftile,
                             dis_rowf.rearrange("o c p -> o (c p)"))
        pbc_x = pp_bc.tile([P, N], F32, tag="pbc")
        nc.tensor.matmul(pbc_x, ones1r, xd_row, start=True, stop=True)
        xd_bc = sm.tile([P, N], F32, tag="xd_bc")
        nc.vector.tensor_copy(xd_bc, pbc_x)
        vd_row = row.tile([1, N], F32R, tag="vd_row")
        nc.vector.tensor_mul(vd_row, v_row,
                             dis_rowf.rearrange("o c p -> o (c p)"))
        pbc_v = pp_bc.tile([P, N], F32, tag="pbc")
        nc.tensor.matmul(pbc_v, ones1r, vd_row, start=True, stop=True)

        # --- rc for NEXT step: rc[k] = dis[k] * sum_n S[k,n]*vd[n] ---
        rc_raw = sm.tile([P, NC], F32, tag="rc_raw")
        for c in range(NC):
            dscr = sm.tile([P, N], F32, tag="dscr")
            nc.vector.tensor_mul(dscr, Sf[:, c, :], pbc_v)
            nc.vector.reduce_sum(rc_raw[:, c:c + 1], dscr, axis=AX.X)
        rc = sm.tile([P, NC], F32, tag="rc")
        nc.vector.tensor_mul(rc, rc_raw, dis_c)

        # --- attention exp tile (t>=1): sigT, spre, softmax ---
        if t >= 1:
            sigT = big.tile([P, NC, N], F32R, tag="sigT")
            if not flags["bs"]:
                for c in range(NC):
                    nc.scalar.activation(sigT[:, c, :], prev["fbc"], AF.Sigmoid,
                                         scale=prev["rc"][:, c:c + 1])
            else:
                pre1 = big.tile([P, NC, N], F32, tag="pre1")
                for c in range(NC):
                    nc.vector.tensor_scalar_mul(pre1[:, c, :], prev["fbc"],
                                                prev["rc"][:, c:c + 1])
                pre2 = big.tile([P, NC, N], F32, tag="pre2")
                nc.vector.tensor_add(pre2, pre1, bsT)
                nc.scalar.activation(sigT, pre2, AF.Sigmoid)

            exp_sm = big1.tile([P, NC, N], F32, tag="exp_sm")
            rec = sm.tile([P, NC], F32, tag="rec")
            se = sm.tile([P, NC], F32, tag="se")
            mxn = sm.tile([P, NC], F32, tag="mxn")
            for mc in range(NC):
                ps_s = pp_spre.tile([P, N], F32, tag="spre")
                for kc in range(NC):
                    nc.tensor.matmul(ps_s, sigT[:, kc, mc * P:(mc + 1) * P],
                                     Vs_r[:, kc, :],
                                     start=(kc == 0), stop=(kc == NC - 1))
                mx = sm.tile([P, 1], F32, tag="mx")
                nc.vector.reduce_max(mx, ps_s, axis=AX.X)
                nc.vector.tensor_scalar_mul(mxn[:, mc:mc + 1], mx, -1.0)
                nc.scalar.activation(exp_sm[:, mc, :], ps_s, AF.Exp,
                                     bias=mxn[:, mc:mc + 1], scale=1.0,
                                     accum_out=se[:, mc:mc + 1])
            nc.vector.reciprocal(rec, se)

        # --- GCN layer 1 (rank-2 collapse) ---
        c_raw = sm.tile([P, NC], F32, tag="c_raw")
        if t >= 1:
            for c in range(NC):
                dscr = sm.tile([P, N], F32, tag="dscr")
                nc.vector.tensor_mul(dscr, exp_sm[:, c, :], xd_bc)
                nc.vector.tensor_mul(dscr, Sf[:, c, :], dscr)
                nc.vector.reduce_sum(c_raw[:, c:c + 1], dscr, axis=AX.X)
            # c = c_raw * dis * rec
            dr = sm.tile([P, NC], F32, tag="dr")
            nc.vector.tensor_mul(dr, dis_c, rec)
            c_col = sm.tile([P, NC], F32, tag="c_col")
            nc.vector.tensor_mul(c_col, c_raw, dr)
        else:
            for c in range(NC):
                dscr = sm.tile([P, N], F32, tag="dscr")
                nc.vector.tensor_mul(dscr, Sf[:, c, :], xd_bc)
                nc.vector.reduce_sum(c_raw[:, c:c + 1], dscr, axis=AX.X)
            c_col = sm.tile([P, NC], F32, tag="c_col")
            nc.vector.tensor_mul(c_col, c_raw, dis_c)

        # c as row (per-column transposes to partition 0)
        c_row = row.tile([1, NC, P], F32R, tag="c_row")
        for c in range(NC):
            ptc = pp_m.tile([1, P], F32, tag="pmisc")
            nc.tensor.transpose(ptc, c_col[:, c:c + 1], ident)
            nc.vector.tensor_copy(c_row[:, c, :], ptc)
        c_row1 = c_row.rearrange("o c p -> o (c p)")

        # G1 = H1^T = lrelu(W1 ⊗ c_row [+ b1 ⊗ rs_row]) : [32, 512]
        ps_g1 = pp_m.tile([GH, N], F32, tag="pmisc")
        if not flags["b1"]:
            nc.tensor.matmul(ps_g1, W1row, c_row1, start=True, stop=True)
        else:
            # rs = rowsum(A_att): needs extra reduction path
            rs_raw = sm.tile([P, NC], F32, tag="rs_raw")
            if t >= 1:
                for c in range(NC):
                    dscr = sm.tile([P, N], F32, tag="dscr")
                    nc.vector.tensor_mul(dscr, exp_sm[:, c, :], dis_bc)
                    nc.vector.tensor_mul(dscr, Sf[:, c, :], dscr)
                    nc.vector.reduce_sum(rs_raw[:, c:c + 1], dscr, axis=AX.X)
                rs_col = sm.tile([P, NC], F32, tag="rs_col")
                nc.vector.tensor_mul(rs_col, rs_raw, dr)
            else:
                for c in range(NC):
                    dscr = sm.tile([P, N], F32, tag="dscr")
                    nc.vector.tensor_mul(dscr, Sf[:, c, :], dis_bc)
                    nc.vector.reduce_sum(rs_raw[:, c:c + 1], dscr, axis=AX.X)
                rs_col = sm.tile([P, NC], F32, tag="rs_col")
                nc.vector.tensor_mul(rs_col, rs_raw, dis_c)
            rs_row = row.tile([1, NC, P], F32R, tag="rs_row")
            for c in range(NC):
                ptr = pp_m.tile([1, P], F32, tag="pmisc")
                nc.tensor.transpose(ptr, rs_col[:, c:c + 1], ident)
                nc.vector.tensor_copy(rs_row[:, c, :], ptr)
            nc.tensor.matmul(ps_g1, W1row, c_row1, start=True, stop=False)
            nc.tensor.matmul(ps_g1, b1row,
                             rs_row.rearrange("o c p -> o (c p)"),
                             start=False, stop=True)
        G1 = sm.tile([GH, N], F32R, tag="G1")
        nc.scalar.activation(G1, ps_g1, AF.Lrelu, alpha=0.01)

        # X2 = (H1@W2 [+b2]) * dis_col : [512, 32] f32r
        X2s = sm.tile([P, NC, GH], F32R, tag="X2s")
        for nc_i in range(NC):
            ps_x2 = pp_m.tile([P, GH], F32, tag="pmisc")
            if not flags["b2"]:
                nc.tensor.matmul(ps_x2, G1[:, nc_i * P:(nc_i + 1) * P], W2s,
                                 start=True, stop=True)
            else:
                nc.tensor.matmul(ps_x2, G1[:, nc_i * P:(nc_i + 1) * P], W2s,
                                 start=True, stop=False)
                nc.tensor.matmul(ps_x2, ones1r, b2row, start=False, stop=True)
            nc.vector.tensor_scalar_mul(X2s[:, nc_i, :], ps_x2,
                                        dis_c[:, nc_i:nc_i + 1])
        # Z2^T = X2s^T @ S : [32, 512]; then * dis_row, lrelu
        ps_z2 = pp_g.tile([GH, N], F32, tag="pz")
        for kc in range(NC):
            nc.tensor.matmul(ps_z2, X2s[:, kc, :], S[:, kc, :],
                             start=(kc == 0), stop=(kc == NC - 1))
        pre_g2 = sm.tile([GH, N], F32, tag="pre_g2")
        nc.vector.tensor_mul(pre_g2, ps_z2, dis_bc[0:GH, :])
        G2 = sm.tile([GH, N], F32R, tag="G2")
        nc.scalar.activation(G2, pre_g2, AF.Lrelu, alpha=0.01)

        # X3 = (H2@W3 [+b3]) * dis_col : [512, 64] f32r
        X3s = sm.tile([P, NC, GO], F32R, tag="X3s")
        for nc_i in range(NC):
            ps_x3 = pp_m.tile([P, GO], F32, tag="pmisc")
            if not flags["b3"]:
                nc.tensor.matmul(ps_x3, G2[:, nc_i * P:(nc_i + 1) * P], W3s,
                                 start=True, stop=True)
            else:
                nc.tensor.matmul(ps_x3, G2[:, nc_i * P:(nc_i + 1) * P], W3s,
                                 start=True, stop=False)
                nc.tensor.matmul(ps_x3, ones1r, b3row, start=False, stop=True)
            nc.vector.tensor_scalar_mul(X3s[:, nc_i, :], ps_x3,
                                        dis_c[:, nc_i:nc_i + 1])
        # Z3^T = X3s^T @ S : [64, 512]; * dis_row, lrelu, accumulate Hs
        ps_z3 = pp_g.tile([GO, N], F32, tag="pz")
        for kc in range(NC):
            nc.tensor.matmul(ps_z3, X3s[:, kc, :], S[:, kc, :],
                             start=(kc == 0), stop=(kc == NC - 1))
        pre_g3 = sm.tile([GO, N], F32, tag="pre_g3")
        nc.vector.tensor_mul(pre_g3, ps_z3, dis_bc[0:GO, :])
        G3 = sm.tile([GO, N], F32, tag="G3")
        nc.scalar.activation(G3, pre_g3, AF.Lrelu, alpha=0.01)
        nc.vector.tensor_add(HsT, HsT, G3)

        # --- LSTM step t ---
        nc.vector.tensor_copy(RCAT[0:1, :], f_r)
        ps_gab = pp_g.tile([P, 2, N], F32, tag="pga")
        nc.tensor.matmul(ps_gab[:, 0, :], wcat_r[:, 0, :], RCAT,
                         start=True, stop=True)
        nc.tensor.matmul(ps_gab[:, 1, :], wcat_r[:, 1, :], RCAT,
                         start=True, stop=True)
        si = sm.tile([LH, N], F32, tag="si")        # sigmoid(i)
        nc.scalar.activation(si, ps_gab[0:LH, 0, :], AF.Sigmoid)
        sf = sm.tile([LH, N], F32, tag="sf")        # sigmoid(f)
        nc.scalar.activation(sf, ps_gab[LH:2 * LH, 0, :], AF.Sigmoid)
        tg = sm.tile([LH, N], F32, tag="tg")        # tanh(g)
        nc.scalar.activation(tg, ps_gab[0:LH, 1, :], AF.Tanh)
        so = sm.tile([LH, N], F32, tag="so")        # sigmoid(o)
        nc.scalar.activation(so, ps_gab[LH:2 * LH, 1, :], AF.Sigmoid)
        t1 = sm.tile([LH, N], F32, tag="t1")
        nc.vector.tensor_mul(t1, sf, c_st)                  # f*c
        t2 = sm.tile([LH, N], F32, tag="t2")
        nc.vector.tensor_mul(t2, si, tg)                    # i*tanh(g)
        nc.vector.tensor_add(c_st, t1, t2)
        tc_ = sm.tile([LH, N], F32, tag="tc_")
        nc.scalar.activation(tc_, c_st, AF.Tanh)
        nc.vector.tensor_mul(RCAT[64:64 + LH, :], so, tc_)  # h -> RCAT rows 64..127

        prev = dict(fbc=fbc_sb, rc=rc)

    # ---------------- head ----------------
    FoT = st.tile([COMB, N], F32R)
    nc.scalar.activation(FoT[0:GO, :], HsT, AF.Copy, scale=1.0 / T)
    nc.vector.tensor_copy(FoT[GO:COMB, :], RCAT.bitcast(F32)[64:64 + LH, :])
    FoTf = FoT.bitcast(F32)

    # R = tanh(Fo @ W_attn [+ b_attn]) ; natural [n, c]
    Ro = st.tile([P, NC, COMB], F32)
    mxh = sm.tile([P, NC], F32, tag="mxh")
    seh = sm.tile([P, NC], F32, tag="seh")
    for nc_i in range(NC):
        ps_r = pp_m.tile([P, COMB], F32, tag="pmisc")
        if not flags["b_attn"]:
            nc.tensor.matmul(ps_r, FoT[:, nc_i * P:(nc_i + 1) * P], Wattn_r,
                             start=True, stop=True)
        else:
            nc.tensor.matmul(ps_r, FoT[:, nc_i * P:(nc_i + 1) * P], Wattn_r,
                             start=True, stop=False)
            nc.tensor.matmul(ps_r, ones1r, battnrow, start=False, stop=True)
        th = sm.tile([P, COMB], F32, tag="th")
        nc.scalar.activation(th, ps_r, AF.Tanh)
        mx1 = sm.tile([P, 1], F32, tag="mx1h")
        nc.vector.reduce_max(mx1, th, axis=AX.X)
        nc.vector.tensor_scalar_mul(mxh[:, nc_i:nc_i + 1], mx1, -1.0)
        nc.scalar.activation(Ro[:, nc_i, :], th, AF.Exp,
                             bias=mxh[:, nc_i:nc_i + 1], scale=1.0,
                             accum_out=seh[:, nc_i:nc_i + 1])
    rech = sm.tile([P, NC], F32, tag="rech")
    nc.vector.reciprocal(rech, seh)
    # normalize Ro (exp/sum), transpose to RoT, Fa^T = FoT ⊙ RoT
    RoT = st.tile([COMB, N], F32)
    for nc_i in range(NC):
        nc.vector.tensor_scalar_mul(Ro[:, nc_i, :], Ro[:, nc_i, :],
                                    rech[:, nc_i:nc_i + 1])
        ptr2 = pp_m.tile([COMB, P], F32, tag="pmisc")
        nc.tensor.transpose(ptr2, Ro[:, nc_i, :], ident)
        nc.vector.tensor_copy(RoT[:, nc_i * P:(nc_i + 1) * P], ptr2)
    FaT = st.tile([COMB, N], F32R)
    nc.vector.tensor_mul(FaT, FoTf, RoT)

    # Ho1 = lrelu(adjn @ (Fa@Wh1 [+bh1])): X [512,32] -> Z^T [32,512]
    X1h = sm.tile([P, NC, HYB1], F32R, tag="X1h")
    for nc_i in range(NC):
        ps_x = pp_m.tile([P, HYB1], F32, tag="pmisc")
        if not flags["bh1"]:
            nc.tensor.matmul(ps_x, FaT[:, nc_i * P:(nc_i + 1) * P], Wh1_r,
                             start=True, stop=True)
        else:
            nc.tensor.matmul(ps_x, FaT[:, nc_i * P:(nc_i + 1) * P], Wh1_r,
                             start=True, stop=False)
            nc.tensor.matmul(ps_x, ones1r, bh1row, start=False, stop=True)
        nc.vector.tensor_copy(X1h[:, nc_i, :], ps_x)
    ps_z1h = pp_g.tile([HYB1, N], F32, tag="pz")
    for kc in range(NC):
        nc.tensor.matmul(ps_z1h, X1h[:, kc, :], adjn_r[:, kc, :],
                         start=(kc == 0), stop=(kc == NC - 1))
    G1h = st.tile([HYB1, N], F32R)
    nc.scalar.activation(G1h, ps_z1h, AF.Lrelu, alpha=0.01)

    # Ho = lrelu(adjn @ (Ho1@Wh2 [+bh2])): [512,16] -> [16,512]
    X2h = sm.tile([P, NC, HYB2], F32R, tag="X2h")
    for nc_i in range(NC):
        ps_x = pp_m.tile([P, HYB2], F32, tag="pmisc")
        if not flags["bh2"]:
            nc.tensor.matmul(ps_x, G1h[:, nc_i * P:(nc_i + 1) * P], Wh2_r,
                             start=True, stop=True)
        else:
            nc.tensor.matmul(ps_x, G1h[:, nc_i * P:(nc_i + 1) * P], Wh2_r,
                             start=True, stop=False)
            nc.tensor.matmul(ps_x, ones1r, bh2row, start=False, stop=True)
        nc.vector.tensor_copy(X2h[:, nc_i, :], ps_x)
    ps_z2h = pp_g.tile([HYB2, N], F32, tag="pz")
    for kc in range(NC):
        nc.tensor.matmul(ps_z2h, X2h[:, kc, :], adjn_r[:, kc, :],
                         start=(kc == 0), stop=(kc == NC - 1))
    G2h = st.tile([HYB2, N], F32R)
    nc.scalar.activation(G2h, ps_z2h, AF.Lrelu, alpha=0.01)

    # out = Ho @ W_fc [+ b_fc]  : [512, 3]
    out_sb = st.tile([P, NC, PRED], F32)
    for nc_i in range(NC):
        ps_o = pp_m.tile([P, PREDP], F32, tag="pmisc")
        if not flags["b_fc"]:
            nc.tensor.matmul(ps_o, G2h[:, nc_i * P:(nc_i + 1) * P], Wfc_r,
                             start=True, stop=True)
        else:
            nc.tensor.matmul(ps_o, G2h[:, nc_i * P:(nc_i + 1) * P], Wfc_r,
                             start=True, stop=False)
            nc.tensor.matmul(ps_o, ones1r, bfcrow4, start=False, stop=True)
        nc.vector.tensor_copy(out_sb[:, nc_i, :], ps_o[:, 0:PRED])
    for c in range(NC):
        nc.sync.dma_start(
            out=out_d[:, c * P:(c + 1) * P].rearrange("p q -> q p"),
            in_=out_sb[:, c, :])

    ctx.close()
    tc.__exit__(None, None, None)
    return nc


_cache = {}


def _get_nc(flags):
    key = tuple(sorted(flags.items()))
    if key not in _cache:
        nc = _emit_wrapper(flags)
        _cache[key] = nc
    return _cache[key]


def _emit_wrapper(flags):
    nc = bass.Bass()
    _emit(nc, flags)
    _split_excess_waits(nc)
    return nc


def kernel(**inputs):
    x = inputs["x"]
    B = x.shape[0]
    assert x.shape == (8, T, N, 1)
    flags = {k: bool(np.any(inputs[k])) for k in
             ["bs", "b1", "b2", "b3", "b_attn", "bh1", "bh2", "b_fc"]}
    nc = _get_nc(flags)

    shared = dict(
        adjm=np.ascontiguousarray(inputs["adj_mask"], np.float32),
        adjn=np.ascontiguousarray(inputs["adj_norm"], np.float32),
        Vs=np.ascontiguousarray(inputs["Vs"], np.float32),
        Ws=np.ascontiguousarray(inputs["Ws"], np.float32),
        Wp=np.ascontiguousarray(inputs["Wp"], np.float32),
        Wa=np.ascontiguousarray(inputs["Wa"], np.float32),
        W1=np.ascontiguousarray(inputs["W1"], np.float32),
        W2=np.ascontiguousarray(inputs["W2"], np.float32),
        W3=np.ascontiguousarray(inputs["W3"], np.float32),
        W_ih=np.ascontiguousarray(inputs["W_ih"], np.float32),
        W_hh=np.ascontiguousarray(inputs["W_hh"], np.float32),
        b_ih=np.ascontiguousarray(inputs["b_ih"], np.float32),
        b_hh=np.ascontiguousarray(inputs["b_hh"], np.float32),
        W_attn=np.ascontiguousarray(inputs["W_attn"], np.float32),
        Wh1=np.ascontiguousarray(inputs["Wh1"], np.float32),
        Wh2=np.ascontiguousarray(inputs["Wh2"], np.float32),
        W_fc=np.ascontiguousarray(inputs["W_fc"], np.float32),
    )
    for k_src, k_dst in [("bs", "bs"), ("b1", "b1"), ("b2", "b2"),
                         ("b3", "b3"), ("b_attn", "b_attn"), ("bh1", "bh1"),
                         ("bh2", "bh2"), ("b_fc", "b_fc")]:
        if flags[k_dst]:
            shared[k_dst] = np.ascontiguousarray(inputs[k_src], np.float32)

    in_maps = []
    for b in range(B):
        m = dict(shared)
        m["xb"] = np.ascontiguousarray(x[b, :, :, 0], np.float32)
        in_maps.append(m)

    res = run_bass_kernel_spmd(nc, in_maps, core_ids=list(range(8)))
    out = np.stack([res.results[b]["out"] for b in range(B)], axis=0)
    return out[..., None].astype(np.float32)


def run_traced(inputs):
    """Like kernel() but with trace=True; returns (out, BassKernelResults)."""
    x = inputs["x"]
    flags = {k: bool(np.any(inputs[k])) for k in
             ["bs", "b1", "b2", "b3", "b_attn", "bh1", "bh2", "b_fc"]}
    nc = _get_nc(flags)
    shared = {n: np.ascontiguousarray(inputs[{
        "adjm": "adj_mask", "adjn": "adj_norm"}.get(n, n)], np.float32)
        for n in ["Vs", "Ws", "Wp", "Wa", "W1", "W2", "W3", "W_ih", "W_hh",
                  "b_ih", "b_hh", "W_attn", "Wh1", "Wh2", "W_fc",
                  "adjm", "adjn"]}
    for k in ["bs", "b1", "b2", "b3", "b_attn", "bh1", "bh2", "b_fc"]:
        if flags[k]:
            shared[k] = np.ascontiguousarray(inputs[k], np.float32)
    in_maps = []
    for b in range(8):
        m = dict(shared)
        m["xb"] = np.ascontiguousarray(x[b, :, :, 0], np.float32)
        in_maps.append(m)
    res = run_bass_kernel_spmd(nc, in_maps, core_ids=list(range(8)),
                               trace=True)
    out = np.stack([res.results[b]["out"] for b in range(8)], axis=0)
    return out[..., None].astype(np.float32), res
